# revision 1
# baseline (speedup 1.0000x reference)
"""Trainium2 Bass kernel for CompleteW2MLSupConLoss.

Strategy (8 NeuronCores, SPMD):
  * Host sorts rows by label (stable) and hands every core the full sorted
    feature/label arrays ROTATED so that core c's 1024 anchor rows sit at
    positions [0, 1024).  One identical program runs on all cores; only the
    data differs.  The scalar loss is permutation-invariant, so no unpermute
    is needed -- each core returns two partial sums which the host combines.
  * Sorting makes the positive-pair mask block diagonal: for anchor row-tile
    t (128 rows) all positives live in columns [128t-pad, 128t+128+pad) where
    pad = max_class_count - 1.  The positive-side work (weights, masked sums)
    therefore runs on a narrow window instead of the full 8192 columns.
  * Dense per-tile work is only: 2 accumulating fp32 matmuls (PE), one ACT
    Exp pass with fused row-sum (softmax denominator), and one custom DVE op
      out = (relu(in0*c0 + c1) + c2) * in1,  accum_out = rowsum(out)
    which computes the W2ML negative weight times exp in a single pass.
  * The self-similarity diagonal is excluded exactly: the diagonal 128-col
    segment of the Exp tile is zeroed on the diagonal (multiply by 1-eye with
    fused row-sum) and s_ii is extracted exactly via an eye-masked reduce.

Math (row i, sums over j != i, T = temperature):
  e_ij   = exp((s_ij - 1)/T)          (shift by 1 ~ rowmax; cancels exactly)
  denom  = sum_j e_ij
  wp     = 1 + relu(0.5 - s)          (positive hard-mining weight)
  wn     = 1 + relu((s - 0.3)/0.7)    (negative hard-mining weight)
  A      = sum_{pos j} wp             PS = sum_{pos j} wp*s
  possum = (PS - A)/T - log(denom)*A
  E      = sum_j wn*e - sum_{pos j} wn*e
  negsum = E / denom
  out0   = sum_i possum_i / max(pos_cnt_i, 1)
  out1   = sum_i negsum_i / max(neg_cnt_i, 1)
  loss   = -out0/B + 0.3 * out1/B
"""

import numpy as np
from contextlib import ExitStack

# ---- problem constants (hardcoded per contest contract) --------------------
B_FULL = 8192
D_FEAT = 256
N_CORES = 8
TEMPERATURE = 0.07
THR_POS = 0.5
THR_NEG = 0.3
NEG_LOSS_W = 0.3
CT = 1024  # columns per PSUM sim tile (2 banks; fp32 matmuls emitted per 512)
PT = 128  # partition tile

_prog_cache: dict = {}
LAST_RESULTS = None  # BassKernelResults of the most recent HW run (for test.py)


# ---- custom DVE op ---------------------------------------------------------
def _w2ml_op():
    """(relu(in0*c0 + c1) + c2) * in1 with fused add-reduction.

    Used with (c0=1/0.7, c1=-0.3/0.7, c2=1) for the dense negative pass
    (in0 = sim from PSUM, in1 = exp tile) and with (c0=-1, c1=0.5, c2=1) for
    the windowed positive pass (in1 = positive mask).
    """
    import concourse.dve_ops as dve_ops
    from concourse.dve_spec import Spec, Src0, Src1, C0, C1, C2, Zero, relu, lower, _has_src1
    from concourse.dve_uop import DveOpSpec

    name = "W2ML_WMUL_ANT"
    for op in dve_ops.OPS:
        if op.name == name:
            return op

    def _ref(in0, in1, c0, c1, c2):
        b = ((np.maximum(in0.astype(np.float32) * c0 + c1, 0.0) + c2) * in1).astype(
            np.float32
        )
        return b, b.reshape(b.shape[0], -1).sum(axis=-1, keepdims=True)

    from operator import add

    spec = Spec(body=(relu(Src0 * C0 + C1) + C2) * Src1, accum=add,
                accum_init=Zero, reference=_ref)
    shas = {}
    for ver in ("v3", "v4"):
        try:
            uops = lower(spec, ver=ver)
            shas[ver] = DveOpSpec(name=name, opcode=None, uops=uops,
                                  rd1_en=_has_src1(spec)).sha(ver)
        except Exception:
            pass
    op = dve_ops.DveOp(name, spec, subdim=False, uops_sha=shas)
    row = max(dve_ops._SUB_OPCODE_FOR_NAME.values()) + 1
    assert row < 0x20
    dve_ops.OPS.append(op)
    dve_ops.CUSTOM_DVE_SPECS[name] = spec
    dve_ops._SUB_OPCODE_FOR_NAME[name] = row
    return op


# ---- window geometry (host side) ------------------------------------------
def _window_pieces(t, pad, b_cols):
    """Column pieces [(ct, lo, hi)] of window [128t-pad, 128t+128+pad) mod B."""
    wlo = PT * t - pad
    whi = PT * t + PT + pad
    if whi - wlo >= b_cols:
        segs = [(0, b_cols)]
    elif wlo < 0:
        segs = [(b_cols + wlo, b_cols), (0, whi)]
    elif whi > b_cols:
        segs = [(wlo, b_cols), (0, whi - b_cols)]
    else:
        segs = [(wlo, whi)]
    pieces = []
    for s0, s1 in segs:
        ct0, ct1 = s0 // CT, (s1 - 1) // CT
        for ct in range(ct0, ct1 + 1):
            lo = max(s0, ct * CT) - ct * CT
            hi = min(s1, (ct + 1) * CT) - ct * CT
            if hi > lo:
                pieces.append((ct, lo, hi))
    return pieces


# ---- program builder -------------------------------------------------------
def _build(b_cols, r_rows, pad, reps=1, loop_n=1, stage="full"):
    """Build+compile the per-core Bass program. r_rows = anchor rows per core.

    reps > 1 re-emits the compute phases back-to-back inside one NEFF; used
    only for amortized device-time measurement (results unchanged)."""
    import concourse.bass as bass
    import concourse.mybir as mybir
    import concourse.tile as tile
    from concourse import bacc

    op = _w2ml_op()
    f32 = mybir.dt.float32
    AF = mybir.ActivationFunctionType
    ALU = mybir.AluOpType
    AX = mybir.AxisListType

    KB = D_FEAT // PT          # 2 contraction blocks
    NT_F = b_cols // PT        # feature row tiles (64)
    RT = r_rows // PT          # anchor row tiles per core (8)
    NCT = b_cols // CT         # 16 column tiles
    invT = 1.0 / TEMPERATURE

    all_pieces = [_window_pieces(t, pad, b_cols) for t in range(RT)]
    npmax = max(len(p) for p in all_pieces)
    wmax = min(CT, PT + 2 * pad)

    nc = bacc.Bacc("TRN2", target_bir_lowering=False, debug=False,
                   num_devices=N_CORES)
    ft_dram = nc.dram_tensor("ft", [D_FEAT, b_cols], f32, kind="ExternalInput").ap()
    lab_dram = nc.dram_tensor("lab", [b_cols], f32, kind="ExternalInput").ap()
    eye_dram = nc.dram_tensor("eye", [PT, PT], f32, kind="ExternalInput").ap()
    ieye_dram = nc.dram_tensor("ieye", [PT, PT], f32, kind="ExternalInput").ap()
    out_dram = nc.dram_tensor("out", [1, 2], f32, kind="ExternalOutput").ap()

    with tile.TileContext(nc) as tc, ExitStack() as ctx:
        singles = ctx.enter_context(tc.tile_pool(name="singles", bufs=1))
        spsum = ctx.enter_context(tc.tile_pool(name="spsum", bufs=2, space="PSUM"))
        rpsum = ctx.enter_context(tc.tile_pool(name="rpsum", bufs=1, space="PSUM"))
        epool = ctx.enter_context(tc.tile_pool(name="epool", bufs=3))
        t5pool = ctx.enter_context(tc.tile_pool(name="t5pool", bufs=3))
        accpool = ctx.enter_context(tc.tile_pool(name="accpool", bufs=2))
        wpool = ctx.enter_context(tc.tile_pool(name="wpool", bufs=3))
        lrpool = ctx.enter_context(tc.tile_pool(name="lrpool", bufs=2))

        aT = singles.tile([PT, KB, b_cols], f32)       # normalized features^T
        eye = singles.tile([PT, PT], f32)
        ieye = singles.tile([PT, PT], f32)
        nc.sync.dma_start(eye, eye_dram)
        nc.sync.dma_start(ieye, ieye_dram)
        zb = singles.tile([PT, 1], f32)
        nc.vector.memset(zb, 0.0)
        eb = singles.tile([PT, 1], f32)   # Exp bias = -1/T
        nc.vector.memset(eb, -invT)
        ones_r = singles.tile([1, PT], f32)  # K=1 lhsT for rnorm broadcast
        nc.vector.memset(ones_r, 1.0)

        # per-row-tile result columns
        denom_all = singles.tile([PT, RT], f32)
        st5_all = singles.tile([PT, RT], f32)
        pc_all = singles.tile([PT, RT], f32)
        A_all = singles.tile([PT, RT], f32)
        PS_all = singles.tile([PT, RT], f32)
        MWE_all = singles.tile([PT, RT], f32)
        sdiag_all = singles.tile([PT, RT], f32)

        _loopctx = tc.For_i(0, loop_n, 1) if loop_n > 1 else None
        if _loopctx is not None:
            _loopctx.__enter__()
        for _rep in range(reps):
            # ---- phase 1: row norms on ACT; rnorm broadcast via K=1 PE matmul;
            # aT = fT * rnorm, all chunked so phase 2 can start early ------------
            with ExitStack() as p1ctx:
                fch = p1ctx.enter_context(tc.tile_pool(name=f"fch_{_rep}", bufs=3))
                grp = p1ctx.enter_context(tc.tile_pool(name=f"grp_{_rep}", bufs=2,
                                                       space="PSUM"))
                rro = p1ctx.enter_context(tc.tile_pool(name=f"rro_{_rep}", bufs=2))
                ph1s = p1ctx.enter_context(tc.tile_pool(name=f"ph1s_{_rep}", bufs=4))
                rbp = p1ctx.enter_context(tc.tile_pool(name=f"rbp_{_rep}", bufs=1,
                                                       space="PSUM"))
                UPC = CT // PT                       # u-tiles per chunk
                for cc in range(b_cols // CT):
                    cs = slice(cc * CT, (cc + 1) * CT)
                    ftc = fch.tile([PT, KB, CT], f32, tag="ftc")
                    for k in range(KB):
                        eng = nc.sync if (cc % 2 == 0) else nc.scalar
                        eng.dma_start(ftc[:, k, :], ft_dram[k * PT:(k + 1) * PT, cs])
                    rrow = rro.tile([1, CT], f32, tag="rrow")
                    for uu in range(UPC):
                        us = slice(uu * PT, (uu + 1) * PT)
                        gps = grp.tile([PT, PT], f32, tag="gps")
                        for k in range(KB):
                            nc.tensor.matmul(gps, ftc[:, k, us], ftc[:, k, us],
                                             start=(k == 0), stop=(k == KB - 1))
                        dsc = ph1s.tile([PT, PT], f32, tag="dsc")
                        ss = ph1s.tile([PT, 1], f32, tag="ss")
                        nc.vector.scalar_tensor_tensor(dsc, gps, 0.0, eye,
                                                       ALU.bypass, ALU.mult,
                                                       accum_out=ss)
                        nrm = ph1s.tile([PT, 1], f32, tag="nrm")
                        nc.scalar.activation(nrm, ss, AF.Sqrt, bias=zb)
                        rn = ph1s.tile([PT, 1], f32, tag="rn")
                        nc.vector.reciprocal(rn, nrm)
                        nc.gpsimd.dma_start(rrow[0:1, us], rn)
                    for h in range(CT // 512):
                        hs = slice(h * 512, (h + 1) * 512)
                        rb = rbp.tile([PT, 512], f32, tag="rb")
                        nc.tensor.matmul(rb, ones_r, rrow[0:1, hs],
                                         start=True, stop=True)
                        for k in range(KB):
                            nc.vector.tensor_mul(
                                aT[:, k, cc * CT + h * 512:cc * CT + (h + 1) * 512],
                                ftc[:, k, hs], rb)

            # ---- phase 2: main sweep -------------------------------------------
            for t in range(RT):
                pieces = all_pieces[t]
                dct, da = (PT * t) // CT, (PT * t) % CT
                l_row = lrpool.tile([PT, 1], f32)
                nc.gpsimd.dma_start(
                    l_row, lab_dram[PT * t:PT * (t + 1)].rearrange("(p o) -> p o", o=1))

                acc_e = accpool.tile([PT, NCT + 2], f32, tag="acc_e")
                nc.vector.memset(acc_e, 0.0)
                acc_t5 = accpool.tile([PT, NCT + 2], f32, tag="acc_t5")
                nc.vector.memset(acc_t5, 0.0)
                acc_pc = accpool.tile([PT, npmax], f32, tag="acc_pc")
                acc_A = accpool.tile([PT, npmax], f32, tag="acc_A")
                acc_PS = accpool.tile([PT, npmax], f32, tag="acc_PS")
                acc_MWE = accpool.tile([PT, npmax], f32, tag="acc_MWE")
                for a in (acc_pc, acc_A, acc_PS, acc_MWE):
                    nc.vector.memset(a, 0.0)

                for ct in range(NCT):
                    ps = spsum.tile([PT, CT], f32)
                    for k in range(KB):
                        for h in range(CT // 512):
                            nc.tensor.matmul(
                                ps[:, h * 512:(h + 1) * 512],
                                aT[:, k, PT * t:PT * (t + 1)],
                                aT[:, k, CT * ct + h * 512:CT * ct + (h + 1) * 512],
                                start=(k == 0), stop=(k == KB - 1))
                    et = epool.tile([PT, CT], f32)
                    t5 = t5pool.tile([PT, CT], f32)
                    if stage == "mm":
                        continue
                    if ct == dct:
                        # split Exp and the custom pass around the diagonal block
                        if da > 0:
                            nc.scalar.activation(et[:, :da], ps[:, :da], AF.Exp,
                                                 bias=eb, scale=invT,
                                                 accum_out=acc_e[:, ct:ct + 1])
                        if da > 0 and stage != "nodve":
                            nc.vector._custom_dve(op, out=t5[:, :da], in0=ps[:, :da],
                                                  in1=et[:, :da],
                                                  s0=1.0 / (1.0 - THR_NEG),
                                                  s1=-THR_NEG / (1.0 - THR_NEG),
                                                  imm2=1.0,
                                                  accum_out=acc_t5[:, ct:ct + 1])
                        dsl = slice(da, da + PT)
                        nc.scalar.activation(et[:, dsl], ps[:, dsl], AF.Exp,
                                             bias=eb, scale=invT)
                        # zero the diagonal into et (scratch out), fused row-sum
                        esc = wpool.tile([PT, PT], f32, tag="esc")
                        nc.vector.scalar_tensor_tensor(esc, et[:, dsl], 0.0, ieye,
                                                       ALU.bypass, ALU.mult,
                                                       accum_out=acc_e[:, NCT:NCT + 1])
                        if stage != "nodve":
                            nc.vector._custom_dve(op, out=t5[:, dsl], in0=ps[:, dsl],
                                              in1=esc,
                                              s0=1.0 / (1.0 - THR_NEG),
                                              s1=-THR_NEG / (1.0 - THR_NEG), imm2=1.0,
                                              accum_out=acc_t5[:, NCT:NCT + 1])
                        if da + PT < CT:
                            psl = slice(da + PT, CT)
                            nc.scalar.activation(et[:, psl], ps[:, psl], AF.Exp,
                                                 bias=eb, scale=invT,
                                                 accum_out=acc_e[:, NCT + 1:NCT + 2])
                        if da + PT < CT and stage != "nodve":
                            nc.vector._custom_dve(op, out=t5[:, psl], in0=ps[:, psl],
                                                  in1=et[:, psl],
                                                  s0=1.0 / (1.0 - THR_NEG),
                                                  s1=-THR_NEG / (1.0 - THR_NEG),
                                                  imm2=1.0,
                                                  accum_out=acc_t5[:, NCT + 1:NCT + 2])
                        # exact diagonal similarity s_ii
                        dsc = wpool.tile([PT, PT], f32, tag="dscr")
                        nc.vector.scalar_tensor_tensor(dsc, ps[:, dsl], 0.0, eye,
                                                       ALU.bypass, ALU.mult,
                                                       accum_out=sdiag_all[:, t:t + 1])
                    else:
                        nc.scalar.activation(et, ps, AF.Exp, bias=eb, scale=invT,
                                             accum_out=acc_e[:, ct:ct + 1])
                        if stage == "nodve":
                            continue
                        nc.vector._custom_dve(op, out=t5, in0=ps, in1=et,
                                              s0=1.0 / (1.0 - THR_NEG),
                                              s1=-THR_NEG / (1.0 - THR_NEG), imm2=1.0,
                                              accum_out=acc_t5[:, ct:ct + 1])
                    for pidx, (pct, lo, hi) in enumerate(pieces):
                        if pct != ct or stage != "full":
                            continue
                        w = hi - lo
                        labw = wpool.tile([PT, wmax], f32, tag="labw")
                        nc.gpsimd.dma_start(
                            labw[:, :w],
                            bass.AP(tensor=lab_dram.tensor,
                                    offset=lab_dram.offset + ct * CT + lo,
                                    ap=[[0, PT], [1, w]]))
                        m_p = wpool.tile([PT, wmax], f32, tag="m_p")
                        nc.vector.tensor_scalar(m_p[:, :w], labw[:, :w], l_row, None,
                                                ALU.is_equal, ALU.add,
                                                accum_out=acc_pc[:, pidx:pidx + 1])
                        mwp = wpool.tile([PT, wmax], f32, tag="mwp")
                        nc.vector._custom_dve(op, out=mwp[:, :w], in0=ps[:, lo:hi],
                                              in1=m_p[:, :w], s0=-1.0, s1=THR_POS,
                                              imm2=1.0,
                                              accum_out=acc_A[:, pidx:pidx + 1])
                        scr = wpool.tile([PT, wmax], f32, tag="scr")
                        nc.vector.scalar_tensor_tensor(scr[:, :w], mwp[:, :w], 0.0,
                                                       ps[:, lo:hi], ALU.bypass,
                                                       ALU.mult,
                                                       accum_out=acc_PS[:, pidx:pidx + 1])
                        scr2 = wpool.tile([PT, wmax], f32, tag="scr2")
                        nc.vector.scalar_tensor_tensor(scr2[:, :w], m_p[:, :w], 0.0,
                                                       t5[:, lo:hi], ALU.bypass,
                                                       ALU.mult,
                                                       accum_out=acc_MWE[:, pidx:pidx + 1])

                nc.vector.reduce_sum(denom_all[:, t:t + 1], acc_e, axis=AX.X)
                nc.vector.reduce_sum(st5_all[:, t:t + 1], acc_t5, axis=AX.X)
                nc.vector.reduce_sum(pc_all[:, t:t + 1], acc_pc, axis=AX.X)
                nc.vector.reduce_sum(A_all[:, t:t + 1], acc_A, axis=AX.X)
                nc.vector.reduce_sum(PS_all[:, t:t + 1], acc_PS, axis=AX.X)
                nc.vector.reduce_sum(MWE_all[:, t:t + 1], acc_MWE, axis=AX.X)


        if _loopctx is not None:
            _loopctx.__exit__(None, None, None)

        # ---- phase 3: per-row scalars + final reduction --------------------
        if stage != "full":
            outs0 = singles.tile([1, 2], f32)
            nc.vector.memset(outs0, 0.0)
            nc.sync.dma_start(out_dram, outs0)
        else:
            fin = singles.tile
            pcm = fin([PT, RT], f32)      # max(pos_cnt, 1)
            nc.vector.tensor_scalar(pcm, pc_all, 1.0, 1.0, ALU.subtract, ALU.max)
            pinv = fin([PT, RT], f32)
            nc.vector.reciprocal(pinv, pcm)
            ncn = fin([PT, RT], f32)      # neg_cnt = B - pc_raw, clipped at 1
            nc.vector.tensor_scalar(ncn, pc_all, -1.0, float(b_cols), ALU.mult, ALU.add)
            nc.vector.tensor_scalar_max(ncn, ncn, 1.0)
            ninv = fin([PT, RT], f32)
            nc.vector.reciprocal(ninv, ncn)
            logden = fin([PT, RT], f32)
            nc.scalar.activation(logden, denom_all, AF.Ln, bias=zb)
            rden = fin([PT, RT], f32)
            nc.vector.reciprocal(rden, denom_all)
            Ac = fin([PT, RT], f32)
            nc.vector.tensor_scalar_sub(Ac, A_all, 1.0)
            PSc = fin([PT, RT], f32)
            nc.vector.tensor_sub(PSc, PS_all, sdiag_all)
            t1 = fin([PT, RT], f32)
            nc.vector.tensor_sub(t1, PSc, Ac)
            t2 = fin([PT, RT], f32)
            nc.vector.tensor_mul(t2, logden, Ac)
            possum = fin([PT, RT], f32)
            nc.vector.scalar_tensor_tensor(possum, t1, invT, t2, ALU.mult, ALU.subtract)
            resv = fin([PT, 2], f32)
            junk1 = fin([PT, RT], f32)
            nc.vector.scalar_tensor_tensor(junk1, possum, 0.0, pinv, ALU.bypass,
                                           ALU.mult, accum_out=resv[:, 0:1])
            E = fin([PT, RT], f32)
            nc.vector.tensor_sub(E, st5_all, MWE_all)
            t4 = fin([PT, RT], f32)
            nc.vector.tensor_mul(t4, E, rden)
            junk2 = fin([PT, RT], f32)
            nc.vector.scalar_tensor_tensor(junk2, t4, 0.0, ninv, ALU.bypass,
                                           ALU.mult, accum_out=resv[:, 1:2])
            ones = fin([PT, 1], f32)
            nc.vector.memset(ones, 1.0)
            psr = rpsum.tile([1, 2], f32)
            nc.tensor.matmul(psr, ones, resv, start=True, stop=True)
            outs = fin([1, 2], f32)
            nc.scalar.copy(outs, psr)
            nc.sync.dma_start(out_dram, outs)

    nc.compile()
    return nc


# ---- host orchestration ----------------------------------------------------
def _prep(features, labels, n_cores):
    features = np.ascontiguousarray(np.asarray(features, dtype=np.float32))
    labels = np.asarray(labels).astype(np.int64)
    b = features.shape[0]
    order = np.argsort(labels, kind="stable")
    f_s = features[order]
    l_s = labels[order].astype(np.float32)
    counts = np.bincount(labels)
    pad = int(max(counts.max() - 1, 0))
    r = b // n_cores
    eye = np.eye(PT, dtype=np.float32)
    ieye = (1.0 - eye).astype(np.float32)
    in_maps = []
    for c in range(n_cores):
        sh = c * r
        f_rot = np.roll(f_s, -sh, axis=0)
        in_maps.append({
            "ft": np.ascontiguousarray(f_rot.T),
            "lab": np.ascontiguousarray(np.roll(l_s, -sh)),
            "eye": eye,
            "ieye": ieye,
        })
    return in_maps, pad, r, b


def _combine(results, b):
    p = sum(float(r["out"][0, 0]) for r in results)
    n = sum(float(r["out"][0, 1]) for r in results)
    loss = -p / b + NEG_LOSS_W * (n / b)
    return np.float32(loss)


def kernel(features, labels):
    global LAST_RESULTS
    from concourse import bass_utils

    in_maps, pad, r, b = _prep(features, labels, N_CORES)
    key = (b, r, pad)
    if key not in _prog_cache:
        _prog_cache[key] = _build(b, r, pad)
    nc = _prog_cache[key]
    res = bass_utils.run_bass_kernel_spmd(nc, in_maps, core_ids=list(range(N_CORES)))
    LAST_RESULTS = res
    return _combine(res.results, b)


def kernel_sim(features, labels, n_cores=N_CORES):
    """CoreSim-backed variant for correctness testing (no hardware)."""
    from concourse.bass_interp import CoreSim

    in_maps, pad, r, b = _prep(features, labels, n_cores)
    nc = _build_for(b, r, pad, n_cores)
    results = []
    for c in range(n_cores):
        sim = CoreSim(nc, trace=False)
        for name, arr in in_maps[c].items():
            sim.tensor(name)[:] = arr
        sim.simulate(check_with_hw=False)
        results.append({"out": np.array(sim.tensor("out"))})
    return _combine(results, b)


def _build_for(b, r, pad, n_cores):
    key = (b, r, pad)
    if key not in _prog_cache:
        _prog_cache[key] = _build(b, r, pad)
    return _prog_cache[key]



# revision 21
# speedup vs baseline: 10.5747x; 10.5747x over previous
"""Trainium2 Bass kernel for CompleteW2MLSupConLoss (v2, bf16).

Strategy (8 NeuronCores, SPMD):
  * Host sorts rows by label (stable) and hands every core the full sorted
    feature/label arrays ROTATED so that core c's 1024 anchor rows sit at
    positions [0, 1024).  One identical program runs on all cores; the scalar
    loss is permutation-invariant, so each core returns two partial sums which
    the host combines.
  * All matmuls run in bf16 (4x the fp32 PE rate); features are shipped as
    bf16 [D, B] so the input DMA is half the fp32 size.  PSUM accumulation
    stays fp32.
  * Per-column norms: sq = Square(fT) on ACT, norm2 = ones-vector matmul on
    PE (partition reduction), ln on ACT, then rn = exp(-0.5*ln) -- both Ln
    and Exp live in the same activation-table set as the dense Exp, so the
    whole kernel uses ONE table set (no ~2.7us table switches).
    rn is staged through a DRAM scratch to transpose partition-major ->
    free-major and then broadcast to [128, CT] via a stride-0 DMA; the
    normalize multiply runs on GPSIMD (bf16, SBUF-only).
  * Dense per-tile work on [128, 2048] PSUM sim tiles: one ACT Exp pass with
    fused row-sum (softmax denominator) and one custom DVE op
      out = (relu(in0*c0 + c1) + c2) * in1,  accum_out = rowsum(out)
    for the W2ML negative-weight times exp.  The self term is NOT masked in
    either pass: it cancels exactly in E = sum(wn*e) - sum_pos(wn*e), and the
    denominator subtracts the exactly-extracted diagonal e_ii.
  * The positive-pair window work (sorted labels => positives within
    [128t-pad, 128t+128+pad)) runs on narrow [128, W] slabs; the is_equal
    mask and the mask*t5 reduction run on GPSIMD, the s-dependent ops on DVE.

Math (row i, sums over j != i, T = temperature):
  e_ij   = exp((s_ij - 1)/T)          (shift by 1; cancels exactly)
  denom  = sum_j e_ij = rowsum(e) - e_ii
  wp     = 1 + relu(0.5 - s)          (positive hard-mining weight)
  wn     = 1 + relu((s - 0.3)/0.7)    (negative hard-mining weight)
  A      = sum_{pos j} wp             PS = sum_{pos j} wp*s
  possum = (PS - A)/T - log(denom)*A
  E      = sum_j wn*e - sum_{pos j} wn*e   (self term cancels)
  negsum = E / denom
  out0   = sum_i possum_i / max(pos_cnt_i, 1)
  out1   = sum_i negsum_i / max(neg_cnt_i, 1)
  loss   = -out0/B + 0.3 * out1/B
"""

import numpy as np
from contextlib import ExitStack

# ---- problem constants (hardcoded per contest contract) --------------------
B_FULL = 8192
D_FEAT = 256
N_CORES = 8
TEMPERATURE = 0.07
THR_POS = 0.5
THR_NEG = 0.3
NEG_LOSS_W = 0.3
CT = 2048  # columns per PSUM sim tile (4 banks; fp32 matmuls emitted per 512)
PT = 128   # partition tile
PADL = 32  # left padding of the window-label staging array

_prog_cache: dict = {}
LAST_RESULTS = None  # BassKernelResults of the most recent HW run (for test.py)


# ---- custom DVE op ---------------------------------------------------------
def _w2ml_op():
    """(relu(in0*c0 + c1) + c2) * in1 with fused add-reduction.

    Used with (c0=1/0.7, c1=-0.3/0.7, c2=1) for the dense negative pass
    (in0 = sim from PSUM, in1 = exp tile) and with (c0=-1, c1=0.5, c2=1) for
    the windowed positive pass (in1 = positive mask).
    """
    import concourse.dve_ops as dve_ops
    from concourse.dve_spec import Spec, Src0, Src1, C0, C1, C2, Zero, relu, lower, _has_src1
    from concourse.dve_uop import DveOpSpec

    name = "W2ML_WMUL_ANT"
    for op in dve_ops.OPS:
        if op.name == name:
            return op

    def _ref(in0, in1, c0, c1, c2):
        b = ((np.maximum(in0.astype(np.float32) * c0 + c1, 0.0) + c2) * in1).astype(
            np.float32
        )
        return b, b.reshape(b.shape[0], -1).sum(axis=-1, keepdims=True)

    from operator import add

    spec = Spec(body=(relu(Src0 * C0 + C1) + C2) * Src1, accum=add,
                accum_init=Zero, reference=_ref)
    shas = {}
    for ver in ("v3", "v4"):
        try:
            uops = lower(spec, ver=ver)
            shas[ver] = DveOpSpec(name=name, opcode=None, uops=uops,
                                  rd1_en=_has_src1(spec)).sha(ver)
        except Exception:
            pass
    op = dve_ops.DveOp(name, spec, subdim=False, uops_sha=shas)
    row = max(dve_ops._SUB_OPCODE_FOR_NAME.values()) + 1
    assert row < 0x20
    dve_ops.OPS.append(op)
    dve_ops.CUSTOM_DVE_SPECS[name] = spec
    dve_ops._SUB_OPCODE_FOR_NAME[name] = row
    return op


# ---- window geometry (host side) ------------------------------------------
def _window_pieces(t, pad, b_cols):
    """Column pieces [(ct, lo, hi)] of window [128t-pad, 128t+128+pad) mod B."""
    wlo = PT * t - pad
    whi = PT * t + PT + pad
    if whi - wlo >= b_cols:
        segs = [(0, b_cols)]
    elif wlo < 0:
        segs = [(b_cols + wlo, b_cols), (0, whi)]
    elif whi > b_cols:
        segs = [(wlo, b_cols), (0, whi - b_cols)]
    else:
        segs = [(wlo, whi)]
    pieces = []
    for s0, s1 in segs:
        ct0, ct1 = s0 // CT, (s1 - 1) // CT
        for ct in range(ct0, ct1 + 1):
            lo = max(s0, ct * CT) - ct * CT
            hi = min(s1, (ct + 1) * CT) - ct * CT
            if hi > lo:
                pieces.append((ct, lo, hi))
    return pieces


def _piece_window_offset(t, pad, b_cols, ct, lo):
    """Offset of piece start inside the window slab [128t-pad, ...)."""
    g = ct * CT + lo
    w0 = PT * t - pad
    off = g - w0
    if off >= b_cols - 2 * pad:  # wrap piece (global col near b_cols)
        off = g - b_cols - w0
    return off


# ---- program builder -------------------------------------------------------
def _build(b_cols, r_rows, pad, reps=1, loop_n=1, stage="full"):
    """Build+compile the per-core Bass program. r_rows = anchor rows per core.

    reps > 1 re-emits the compute phases back-to-back inside one NEFF; used
    only for amortized device-time measurement (results unchanged)."""
    import concourse.bass as bass
    import concourse.mybir as mybir
    import concourse.tile as tile
    from concourse import bacc

    op = _w2ml_op()
    f32 = mybir.dt.float32
    bf16 = mybir.dt.bfloat16
    AF = mybir.ActivationFunctionType
    ALU = mybir.AluOpType
    AX = mybir.AxisListType

    KB = D_FEAT // PT          # 2 contraction blocks
    RT = r_rows // PT          # anchor row tiles per core (8)
    NCT = b_cols // CT         # 4 column tiles
    invT = 1.0 / TEMPERATURE
    W = PT + 2 * pad           # window slab width
    H5 = CT // 512             # 512-wide matmul pieces per sim tile

    all_pieces = [_window_pieces(t, pad, b_cols) for t in range(RT)]
    npmax = max(len(p) for p in all_pieces)

    nc = bacc.Bacc("TRN2", target_bir_lowering=False, debug=False,
                   num_devices=N_CORES)
    ft_dram = nc.dram_tensor("ft", [D_FEAT, b_cols], bf16, kind="ExternalInput").ap()
    lab_dram = nc.dram_tensor("lab", [r_rows], f32, kind="ExternalInput").ap()
    labw_dram = nc.dram_tensor("labw", [2048], f32, kind="ExternalInput").ap()
    eye_dram = nc.dram_tensor("eye", [PT, PT], f32, kind="ExternalInput").ap()
    ieye_dram = nc.dram_tensor("ieye", [PT, PT], f32, kind="ExternalInput").ap()
    out_dram = nc.dram_tensor("out", [1, 2], f32, kind="ExternalOutput").ap()
    nscr = nc.dram_tensor("nscr", [b_cols], f32, kind="Internal").ap()
    rscr = nc.dram_tensor("rscr", [b_cols], bf16, kind="Internal").ap()

    with tile.TileContext(nc) as tc, ExitStack() as ctx:
        singles = ctx.enter_context(tc.tile_pool(name="singles", bufs=1))
        spsum = ctx.enter_context(tc.tile_pool(name="spsum", bufs=2, space="PSUM"))
        epool = ctx.enter_context(tc.tile_pool(name="epool", bufs=4))
        t5pool = ctx.enter_context(tc.tile_pool(name="t5pool", bufs=4))
        wpool = ctx.enter_context(tc.tile_pool(name="wpool", bufs=2))

        aT = singles.tile([PT, KB, b_cols], bf16)      # normalized features^T
        eye = singles.tile([PT, PT], f32)
        nc.sync.dma_start(eye, eye_dram)
        ieye = singles.tile([PT, PT], f32)
        nc.sync.dma_start(ieye, ieye_dram)
        zb = singles.tile([PT, 1], f32)
        nc.vector.memset(zb, 0.0)
        zb1 = singles.tile([1, 1], f32)
        nc.vector.memset(zb1, 0.0)
        eb = singles.tile([PT, 1], f32)   # Exp bias = -1/T
        nc.vector.memset(eb, -invT)
        ones_w = singles.tile([PT, 1], bf16)  # K=128 -> M=1 norm reduction
        nc.vector.memset(ones_w, 1.0)
        ones_f = singles.tile([PT, 1], f32)   # final reduction lhsT
        nc.vector.memset(ones_f, 1.0)

        # anchor labels [p, t] and window label slab [p, t, j]
        l_all = singles.tile([PT, RT], f32)
        nc.gpsimd.dma_start(
            l_all, bass.AP(tensor=lab_dram.tensor, offset=lab_dram.offset,
                           ap=[[1, PT], [PT, RT]]))
        labw_all = singles.tile([PT, RT, W], f32)
        nc.gpsimd.dma_start(
            labw_all, bass.AP(tensor=labw_dram.tensor,
                              offset=labw_dram.offset + PADL - pad,
                              ap=[[0, PT], [PT, RT], [1, W]]))

        # per-row-tile result columns
        denom_all = singles.tile([PT, RT], f32)
        st5_all = singles.tile([PT, RT], f32)
        pc_all = singles.tile([PT, RT], f32)
        A_all = singles.tile([PT, RT], f32)
        PS_all = singles.tile([PT, RT], f32)
        MWE_all = singles.tile([PT, RT], f32)
        sdiag_all = singles.tile([PT, RT], f32)

        # persistent accumulators (reduced at end of each rep)
        # acc_e/acc_t5 columns: 0..NCT-1 per column tile (diag tile uses its
        # column for [0, da)), NCT for the diag-zeroed block, NCT+1 for tail
        acc_e = singles.tile([PT, RT, NCT + 2], f32)
        acc_t5 = singles.tile([PT, RT, NCT + 2], f32)
        acc_pc = singles.tile([PT, RT, npmax], f32)
        acc_A = singles.tile([PT, RT, npmax], f32)
        acc_PS = singles.tile([PT, RT, npmax], f32)
        acc_MWE = singles.tile([PT, RT, npmax], f32)

        _loopctx = tc.For_i(0, loop_n, 1) if loop_n > 1 else None
        if _loopctx is not None:
            _loopctx.__enter__()
        for _rep in range(reps):
            if npmax > 1:
                # t>0 rows only write piece 0; zero the stale tail columns
                for a in (acc_pc, acc_A, acc_PS, acc_MWE):
                    nc.vector.memset(a, 0.0)
            # t=0's diag tile leaves its [0, da)=empty column unwritten
            nc.vector.memset(acc_e, 0.0)
            nc.vector.memset(acc_t5, 0.0)

            # ---- phase 1: per-column rnorm + aT = fT * rnorm, chunked ------
            with ExitStack() as p1ctx:
                fch = p1ctx.enter_context(tc.tile_pool(name=f"fch_{_rep}", bufs=2))
                sqp = p1ctx.enter_context(tc.tile_pool(name=f"sqp_{_rep}", bufs=2))
                nst = p1ctx.enter_context(tc.tile_pool(name=f"nst_{_rep}", bufs=2))
                rbp = p1ctx.enter_context(tc.tile_pool(name=f"rbp_{_rep}", bufs=2))
                for cc in range(NCT):
                    cs = slice(cc * CT, (cc + 1) * CT)
                    ftc = fch.tile([PT, KB, CT], bf16, tag="ftc")
                    for k in range(KB):
                        eng = nc.sync if (cc % 2 == 0) else nc.scalar
                        eng.dma_start(ftc[:, k, :], ft_dram[k * PT:(k + 1) * PT, cs])
                    sq = sqp.tile([PT, KB, CT], bf16, tag="sq")
                    nc.scalar.activation(sq, ftc, AF.Square, bias=zb)
                    nps = spsum.tile([PT, CT], f32, tag="ps")
                    for h in range(H5):
                        hs = slice(h * 512, (h + 1) * 512)
                        for k in range(KB):
                            nc.tensor.matmul(nps[0:1, hs], ones_w, sq[:, k, hs],
                                             start=(k == 0), stop=(k == KB - 1))
                    # rn = exp(-0.5*ln(norm2)); stage through DRAM to
                    # transpose free-major -> partition-major and back
                    nrmln = nst.tile([1, CT], f32, tag="nrmln")
                    nc.scalar.activation(nrmln, nps[0:1, :], AF.Ln, bias=zb1)
                    nc.gpsimd.dma_start(
                        bass.AP(tensor=nscr.tensor, offset=nscr.offset + cc * CT,
                                ap=[[0, 1], [1, CT]]), nrmln)
                    nrmT = nst.tile([PT, CT // PT], f32, tag="nrmT")
                    nc.sync.dma_start(
                        nrmT, bass.AP(tensor=nscr.tensor, offset=nscr.offset + cc * CT,
                                      ap=[[1, PT], [PT, CT // PT]]))
                    rn = nst.tile([PT, CT // PT], bf16, tag="rn")
                    nc.scalar.activation(rn, nrmT, AF.Exp, bias=zb, scale=-0.5)
                    nc.gpsimd.dma_start(
                        bass.AP(tensor=rscr.tensor, offset=rscr.offset + cc * CT,
                                ap=[[1, PT], [PT, CT // PT]]), rn)
                    rb = rbp.tile([PT, CT], bf16, tag="rb")
                    nc.sync.dma_start(
                        rb, bass.AP(tensor=rscr.tensor, offset=rscr.offset + cc * CT,
                                    ap=[[0, PT], [1, CT]]))
                    for k in range(KB):
                        nc.gpsimd.tensor_mul(aT[:, k, cs], ftc[:, k, :], rb)

            # ---- phase 2: main sweep (ct outer so it starts on chunk 0) ----
            for ct in range(NCT):
                for t in range(RT):
                    ps = spsum.tile([PT, CT], f32, tag="ps")
                    for k in range(KB):
                        for h in range(H5):
                            nc.tensor.matmul(
                                ps[:, h * 512:(h + 1) * 512],
                                aT[:, k, PT * t:PT * (t + 1)],
                                aT[:, k, CT * ct + h * 512:CT * ct + (h + 1) * 512],
                                start=(k == 0), stop=(k == KB - 1))
                    et = epool.tile([PT, CT], bf16, tag="et")
                    t5 = t5pool.tile([PT, CT], bf16, tag="t5")
                    s0n, s1n = 1.0 / (1.0 - THR_NEG), -THR_NEG / (1.0 - THR_NEG)
                    if stage == "mm":
                        continue
                    if ct != (PT * t) // CT:
                        nc.scalar.activation(et, ps, AF.Exp, bias=eb, scale=invT,
                                             accum_out=acc_e[:, t, ct:ct + 1])
                        nc.vector._custom_dve(op, out=t5, in0=ps, in1=et,
                                              s0=s0n, s1=s1n, imm2=1.0,
                                              accum_out=acc_t5[:, t, ct:ct + 1])
                    else:
                        # split around the self-diagonal block: zero e_ii so no
                        # giant self term ever enters an accumulator
                        da = PT * t - CT * ct
                        if da > 0:
                            nc.scalar.activation(et[:, :da], ps[:, :da], AF.Exp,
                                                 bias=eb, scale=invT,
                                                 accum_out=acc_e[:, t, ct:ct + 1])
                            nc.vector._custom_dve(op, out=t5[:, :da], in0=ps[:, :da],
                                                  in1=et[:, :da], s0=s0n, s1=s1n,
                                                  imm2=1.0,
                                                  accum_out=acc_t5[:, t, ct:ct + 1])
                        dsl = slice(da, da + PT)
                        nc.scalar.activation(et[:, dsl], ps[:, dsl], AF.Exp,
                                             bias=eb, scale=invT)
                        esc = wpool.tile([PT, PT], f32, tag="esc")
                        nc.vector.scalar_tensor_tensor(esc, et[:, dsl], 0.0, ieye,
                                                       ALU.bypass, ALU.mult,
                                                       accum_out=acc_e[:, t, NCT:NCT + 1])
                        nc.vector._custom_dve(op, out=t5[:, dsl], in0=ps[:, dsl],
                                              in1=esc, s0=s0n, s1=s1n, imm2=1.0,
                                              accum_out=acc_t5[:, t, NCT:NCT + 1])
                        if da + PT < CT:
                            psl = slice(da + PT, CT)
                            nc.scalar.activation(et[:, psl], ps[:, psl], AF.Exp,
                                                 bias=eb, scale=invT,
                                                 accum_out=acc_e[:, t, NCT + 1:NCT + 2])
                            nc.vector._custom_dve(op, out=t5[:, psl], in0=ps[:, psl],
                                                  in1=et[:, psl], s0=s0n, s1=s1n,
                                                  imm2=1.0,
                                                  accum_out=acc_t5[:, t, NCT + 1:NCT + 2])
                        # exact diagonal s_ii
                        dscr = wpool.tile([PT, PT], f32, tag="dscr")
                        nc.vector.scalar_tensor_tensor(dscr, ps[:, dsl], 0.0, eye,
                                                       ALU.bypass, ALU.mult,
                                                       accum_out=sdiag_all[:, t:t + 1])
                    for pidx, (pct, lo, hi) in enumerate(all_pieces[t]):
                        if pct != ct or stage != "full":
                            continue
                        w = hi - lo
                        off = _piece_window_offset(t, pad, b_cols, pct, lo)
                        m_p = wpool.tile([PT, W], f32, tag="m_p")
                        nc.vector.tensor_scalar(m_p[:, :w],
                                                labw_all[:, t, off:off + w],
                                                l_all[:, t:t + 1], None,
                                                ALU.is_equal, ALU.add,
                                                accum_out=acc_pc[:, t, pidx:pidx + 1])
                        mwp = wpool.tile([PT, W], f32, tag="mwp")
                        nc.vector._custom_dve(op, out=mwp[:, :w], in0=ps[:, lo:hi],
                                              in1=m_p[:, :w], s0=-1.0, s1=THR_POS,
                                              imm2=1.0,
                                              accum_out=acc_A[:, t, pidx:pidx + 1])
                        scr = wpool.tile([PT, W], f32, tag="scr")
                        nc.vector.scalar_tensor_tensor(scr[:, :w], mwp[:, :w], 0.0,
                                                       ps[:, lo:hi], ALU.bypass,
                                                       ALU.mult,
                                                       accum_out=acc_PS[:, t, pidx:pidx + 1])
                        scr2 = wpool.tile([PT, W], f32, tag="scr2")
                        nc.vector.scalar_tensor_tensor(scr2[:, :w], m_p[:, :w], 0.0,
                                                       t5[:, lo:hi], ALU.bypass,
                                                       ALU.mult,
                                                       accum_out=acc_MWE[:, t, pidx:pidx + 1])

            # ---- batched reductions over the accumulators ------------------
            nc.vector.reduce_sum(denom_all, acc_e, axis=AX.X)
            nc.vector.reduce_sum(st5_all, acc_t5, axis=AX.X)
            nc.vector.reduce_sum(pc_all, acc_pc, axis=AX.X)
            nc.vector.reduce_sum(A_all, acc_A, axis=AX.X)
            nc.vector.reduce_sum(PS_all, acc_PS, axis=AX.X)
            nc.vector.reduce_sum(MWE_all, acc_MWE, axis=AX.X)

        if _loopctx is not None:
            _loopctx.__exit__(None, None, None)

        # ---- phase 3: per-row scalars + final reduction --------------------
        if stage != "full":
            outs0 = singles.tile([1, 2], f32)
            nc.vector.memset(outs0, 0.0)
            nc.sync.dma_start(out_dram, outs0)
        else:
            fin = singles.tile
            pcm = fin([PT, RT], f32)      # max(pos_cnt, 1)
            nc.vector.tensor_scalar(pcm, pc_all, 1.0, 1.0, ALU.subtract, ALU.max)
            pinv = fin([PT, RT], f32)
            nc.vector.reciprocal(pinv, pcm)
            ncn = fin([PT, RT], f32)      # neg_cnt = B - pc_raw, clipped at 1
            nc.vector.tensor_scalar(ncn, pc_all, -1.0, float(b_cols), ALU.mult, ALU.add)
            nc.vector.tensor_scalar_max(ncn, ncn, 1.0)
            ninv = fin([PT, RT], f32)
            nc.vector.reciprocal(ninv, ncn)
            logden = fin([PT, RT], f32)
            nc.scalar.activation(logden, denom_all, AF.Ln, bias=zb)
            rden = fin([PT, RT], f32)
            nc.vector.reciprocal(rden, denom_all)
            Ac = fin([PT, RT], f32)
            nc.vector.tensor_scalar_sub(Ac, A_all, 1.0)
            PSc = fin([PT, RT], f32)
            nc.vector.tensor_sub(PSc, PS_all, sdiag_all)
            t1 = fin([PT, RT], f32)
            nc.vector.tensor_sub(t1, PSc, Ac)
            t2 = fin([PT, RT], f32)
            nc.vector.tensor_mul(t2, logden, Ac)
            possum = fin([PT, RT], f32)
            nc.vector.scalar_tensor_tensor(possum, t1, invT, t2, ALU.mult, ALU.subtract)
            resv = fin([PT, 2], f32)
            junk1 = fin([PT, RT], f32)
            nc.vector.scalar_tensor_tensor(junk1, possum, 0.0, pinv, ALU.bypass,
                                           ALU.mult, accum_out=resv[:, 0:1])
            E = fin([PT, RT], f32)
            nc.vector.tensor_sub(E, st5_all, MWE_all)
            t4 = fin([PT, RT], f32)
            nc.vector.tensor_mul(t4, E, rden)
            junk2 = fin([PT, RT], f32)
            nc.vector.scalar_tensor_tensor(junk2, t4, 0.0, ninv, ALU.bypass,
                                           ALU.mult, accum_out=resv[:, 1:2])
            psr = spsum.tile([PT, CT], f32, tag="ps")
            nc.tensor.matmul(psr[0:1, 0:2], ones_f, resv, start=True, stop=True)
            outs = fin([1, 2], f32)
            nc.scalar.copy(outs, psr[0:1, 0:2])
            nc.sync.dma_start(out_dram, outs)

    nc.compile()
    return nc


# ---- host orchestration ----------------------------------------------------
def _bf16(a):
    import ml_dtypes
    return np.ascontiguousarray(a.astype(ml_dtypes.bfloat16))


def _prep(features, labels, n_cores):
    features = np.ascontiguousarray(np.asarray(features, dtype=np.float32))
    labels = np.asarray(labels).astype(np.int64)
    b = features.shape[0]
    order = np.argsort(labels, kind="stable")
    f_s = features[order]
    l_s = labels[order].astype(np.float32)
    counts = np.bincount(labels)
    pad = int(max(counts.max() - 1, 0))
    assert PT + 2 * pad + PADL < 2048
    r = b // n_cores
    eye = np.eye(PT, dtype=np.float32)
    in_maps = []
    for c in range(n_cores):
        sh = c * r
        f_rot = np.roll(f_s, -sh, axis=0)
        l_rot = np.roll(l_s, -sh)
        labw = np.empty(2048, np.float32)
        idx = (np.arange(2048) - PADL) % b
        labw[:] = l_rot[idx]
        in_maps.append({
            "ft": _bf16(f_rot.T),
            "lab": np.ascontiguousarray(l_rot[:r]),
            "labw": labw,
            "eye": eye,
            "ieye": (1.0 - eye).astype(np.float32),
        })
    return in_maps, pad, r, b


def _combine(results, b):
    p = sum(float(r["out"][0, 0]) for r in results)
    n = sum(float(r["out"][0, 1]) for r in results)
    loss = -p / b + NEG_LOSS_W * (n / b)
    return np.float32(loss)


def kernel(features, labels):
    global LAST_RESULTS
    from concourse import bass_utils

    in_maps, pad, r, b = _prep(features, labels, N_CORES)
    key = (b, r, pad)
    if key not in _prog_cache:
        _prog_cache[key] = _build(b, r, pad)
    nc = _prog_cache[key]
    res = bass_utils.run_bass_kernel_spmd(nc, in_maps, core_ids=list(range(N_CORES)))
    LAST_RESULTS = res
    return _combine(res.results, b)


def kernel_sim(features, labels, n_cores=N_CORES, cores=None):
    """CoreSim-backed variant for correctness testing (no hardware)."""
    from concourse.bass_interp import CoreSim

    in_maps, pad, r, b = _prep(features, labels, n_cores)
    nc = _build_for(b, r, pad, n_cores)
    results = []
    for c in (cores if cores is not None else range(n_cores)):
        sim = CoreSim(nc, trace=False)
        for name, arr in in_maps[c].items():
            sim.tensor(name)[:] = arr
        sim.simulate(check_with_hw=False)
        results.append({"out": np.array(sim.tensor("out"))})
    return results, in_maps, pad, r, b


def _build_for(b, r, pad, n_cores):
    key = (b, r, pad)
    if key not in _prog_cache:
        _prog_cache[key] = _build(b, r, pad)
    return _prog_cache[key]
